# revision 56
# baseline (speedup 1.0000x reference)
"""GCN layer (X@W -> edge gather/scale -> segment-sum by dest -> +b -> relu)
as a Bass/Tile kernel on 8 Trainium2 NeuronCores.

Strategy (1D node partition, SPMD single program):
  - Nodes sharded 12500/core (destination shard), padded to 100 blocks of
    128.  Each core computes its XW shard with bf16 PE matmuls; four
    per-source-quarter AllGathers replicate the bf16 XW table (rows padded
    to 256B for the gather) into every core's DRAM, pipelined so quarter
    q's collective overlaps the gathers of quarter q-1 and the GEMM of
    later quarters.
  - Edges partitioned by destination shard, bucketed by source quarter and
    grouped by dest block of 128.  Sources are gathered from the quarter
    tables with dma_gather (int16 indices, 256B rows).  The gather is
    descriptor-latency-bound, so gathers spread over all 4 SWDGE queues
    and run unthrottled (32KB descriptor scratch).
  - Dest nodes are permuted across blocks (host-side round-based greedy on
    per-quarter degree vectors) so every (block, quarter) cell fits 4
    chunks of 128 edges: padding drops from ~20% to ~4%.  The host inverts
    the permutation when assembling the output.
  - Per 128-edge chunk a selection matrix S[e, d] = val[e] * (dest[e]==d)
    is built in ONE DVE tensor_scalar op (bf16) from a constant iota tile,
    then PE computes psum[128 dests, 64] += S^T @ G (gathered bf16 rows),
    giving the multiply + segment-sum in one matmul.  +bias and relu (f32)
    on eviction.

All chunk counts are padded to the max over cores so all 8 cores run the
same program (required for the collective / PJRT SPMD launch).
"""

import math
from contextlib import ExitStack

import numpy as np

import concourse.bacc as bacc
import concourse.mybir as mybir
import concourse.tile as tile
from concourse.bass import _add_dep_helper
from concourse.bass_utils import run_bass_kernel_spmd

# Problem constants (hardcoded per contract; kernel.py must be self-contained).
N = 100000
E = 1600000
FIN = 256
FOUT = 64
NCORES = 8

P = 128                      # partitions / block size
SHARD = N // NCORES          # 12500 dest nodes per core
NBLK = 100                   # dest blocks per core (12800 rows: quarter-
                             # aligned so the AllGather splits into 4)
SHARD_PAD = NBLK * P         # 12800 (X zero-padded rows)
TABLE_ROWS = NCORES * SHARD_PAD  # 102400
NBUCKET = 4                  # edge buckets = source-shard quarters; bucket q
                             # only needs quarter-AllGather q (pipelining)
QROWS = SHARD_PAD // NBUCKET     # 3200 padded source rows per quarter
QREAL = SHARD // NBUCKET         # 3125 real source rows per quarter (the
                                 # 75-row pad sits at each quarter's end so
                                 # every bucket gets ~25% of the edges)
WINQ = NCORES * QROWS            # 25600 rows per quarter table (int16 ok)
SB_BLOCKS = 8                # dest blocks per super-batch
NSB = math.ceil(NBLK / SB_BLOCKS)
KH = FIN // P                # 2 contraction halves in the GEMM
NQ = 4                       # SWDGE descriptor queues (rings) for gathers
GDEPTH = 10000               # gather chain throttle depth (disabled: probes
                             # showed 4 rings + 64KB scratch run unthrottled)


def _build_plan(edge_row, edge_col, edge_vals):
    """Host-side edge partition/sort/pad.  Returns the uniform structure
    (shared across cores) + per-core staged arrays."""
    core = edge_row // SHARD
    r_local = edge_row - core * SHARD
    src_core = edge_col // SHARD
    src_local = edge_col - src_core * SHARD
    # bucket = source quarter (QREAL real rows each; the host stages source
    # row r at padded slot q*QROWS + (r - q*QREAL)); row index within
    # quarter table q is src_core * QROWS + that in-quarter offset
    bucket = src_local // QREAL
    idx16 = (src_core * QROWS + (src_local - bucket * QREAL)).astype(np.int16)

    # Balance (blk, bucket) cell counts by permuting each core's dest nodes
    # across blocks (greedy 4-vector LPT).  Chunk capacity is ceil(cell/128)
    # maxed over cores, so balanced cells cut the ~20% padding to a few %.
    # The host inverts the permutation in _assemble; sources are unaffected.
    perms = np.empty((NCORES, SHARD), dtype=np.int64)
    for c in range(NCORES):
        m = core == c
        deg4 = np.zeros((SHARD, NBUCKET), dtype=np.int64)
        np.add.at(deg4, (r_local[m], bucket[m]), 1)
        order = np.argsort(-deg4.sum(axis=1), kind="stable")
        cells = np.zeros((NBLK, NBUCKET), dtype=np.int64)
        fills = np.zeros(NBLK, dtype=np.int64)
        slot = np.empty(SHARD, dtype=np.int64)
        big = 1 << 30
        cap = 4 * P  # hard cell cap: exactly 4 chunks per (blk, bucket)
        rate = deg4.sum(axis=0) / SHARD  # per-bucket mean node degree
        # rounds of NBLK nodes, one per block: fills stay exactly even and
        # every block's bucket profile tracks the uniform trajectory
        for r0 in range(0, SHARD, NBLK):
            rnd = order[r0:r0 + NBLK]
            used = np.zeros(NBLK, dtype=bool)
            ideal = np.outer(fills + 1, rate)
            for n in rnd:
                after = cells + deg4[n]
                cost = ((after - ideal) ** 2).sum(axis=1)
                cost = cost + (after > cap).any(axis=1) * big + used * (4 * big)
                j = int(cost.argmin())
                cells[j] += deg4[n]
                slot[n] = j * P + fills[j]
                fills[j] += 1
                used[j] = True
        perms[c] = slot
    new_r = perms[core, r_local]
    blk = new_r // P
    dest_in_blk = (new_r - blk * P).astype(np.float32)

    # sort edges by (core, blk, bucket)
    order = np.lexsort((bucket, blk, core))
    core_s = core[order]
    blk_s = blk[order]
    bucket_s = bucket[order]
    idx16_s = idx16[order]
    dest_s = dest_in_blk[order]
    val_s = edge_vals[order].astype(np.float32)

    # segment counts per (core, blk, bucket)
    seg_key = (core_s * NBLK + blk_s) * NBUCKET + bucket_s
    counts = np.bincount(seg_key, minlength=NCORES * NBLK * NBUCKET).reshape(
        NCORES, NBLK, NBUCKET
    )
    # uniform capacity (in chunks of 128 edges) per (blk, bucket): max over cores
    chunks_bb = np.ceil(counts / P).astype(np.int64).max(axis=0)  # [NBLK, NBUCKET]
    # blocks holding real output rows need >=1 chunk so their PSUM is reset;
    # pure-padding blocks (rows >= SHARD, dropped by _assemble) may be empty
    # and are skipped in the kernel.
    nblk_real = math.ceil(SHARD / P)
    assert chunks_bb[:nblk_real].sum(axis=1).min() >= 1
    cap_bb = chunks_bb * P

    # ---- static layout ----
    # stream order: (sb, bucket, blk in sb, chunk)
    sb_of_blk = np.arange(NBLK) // SB_BLOCKS
    # chunk columns per (sb, bucket): sum of chunks of its blocks
    # slot offsets for each (blk, bucket) within its (sb, bucket) stream
    slot_off = np.zeros((NBLK, NBUCKET), dtype=np.int64)
    sb_b_len = np.zeros((NSB, NBUCKET), dtype=np.int64)   # slots per (sb, bucket)
    for sb in range(NSB):
        blks = np.where(sb_of_blk == sb)[0]
        for b in range(NBUCKET):
            off = 0
            for bk in blks:
                slot_off[bk, b] = off
                off += cap_bb[bk, b]
            sb_b_len[sb, b] = off
    # global offsets: chunk columns and idx columns per (sb, bucket)
    chunk_col0 = np.zeros((NSB, NBUCKET), dtype=np.int64)
    idx_col0 = np.zeros((NSB, NBUCKET), dtype=np.int64)
    ccur = icur = 0
    for sb in range(NSB):
        for b in range(NBUCKET):
            chunk_col0[sb, b] = ccur
            idx_col0[sb, b] = icur
            ccur += sb_b_len[sb, b] // P
            icur += sb_b_len[sb, b] // 16
    CTOT = ccur   # total chunks per core
    ITOT = icur   # total idx columns per core

    # global slot index for every edge:
    #   slot = global_slot0[sb,b] + slot_off[blk,b] + rank_within_segment
    # where global_slot0 = chunk_col0 * 128
    first_of_seg = np.zeros(NCORES * NBLK * NBUCKET + 1, dtype=np.int64)
    np.cumsum(counts.reshape(-1), out=first_of_seg[1:])
    rank = np.arange(len(core_s)) - first_of_seg[seg_key]
    slot = (
        chunk_col0[sb_of_blk[blk_s], bucket_s] * P
        + slot_off[blk_s, bucket_s]
        + rank
    )

    # ---- per-core staged arrays ----
    idx_streams = np.zeros((NCORES, CTOT * P), dtype=np.int16)
    dest_streams = np.zeros((NCORES, CTOT * P), dtype=np.float32)
    val_streams = np.zeros((NCORES, CTOT * P), dtype=np.float32)
    for c in range(NCORES):
        m = core_s == c
        idx_streams[c, slot[m]] = idx16_s[m]
        dest_streams[c, slot[m]] = dest_s[m]
        val_streams[c, slot[m]] = val_s[m]

    # dest/val DRAM layout [128, CTOT]: chunk j, partition p <- stream[j*128+p]
    dest_np = dest_streams.reshape(NCORES, CTOT, P).transpose(0, 2, 1).copy()
    val_np = val_streams.reshape(NCORES, CTOT, P).transpose(0, 2, 1).copy()

    # idx DRAM layout [128, ITOT] int16: within each (sb,b) segment of the
    # stream, idx i -> partition i%16 (replicated over the 8 groups of 16),
    # column i//16
    idx_np = np.zeros((NCORES, P, ITOT), dtype=np.int16)
    for sb in range(NSB):
        for b in range(NBUCKET):
            L = int(sb_b_len[sb, b])
            if L == 0:
                continue
            s0 = int(chunk_col0[sb, b]) * P
            i0 = int(idx_col0[sb, b])
            seg = idx_streams[:, s0:s0 + L].reshape(NCORES, L // 16, 16)
            seg = seg.transpose(0, 2, 1)  # [NCORES, 16, L//16]
            idx_np[:, :, i0:i0 + L // 16] = np.tile(seg, (1, 8, 1))

    # per-block chunk list: (bucket, j_local_in_gather, global_chunk_col)
    blk_chunks = []
    for bk in range(NBLK):
        sb = int(sb_of_blk[bk])
        lst = []
        for b in range(NBUCKET):
            nch = int(chunks_bb[bk, b])
            j0 = int(slot_off[bk, b]) // P
            c0 = int(chunk_col0[sb, b]) + j0
            for k in range(nch):
                lst.append((b, j0 + k, c0 + k))
        blk_chunks.append(lst)

    struct = dict(
        chunks_bb=chunks_bb, sb_b_len=sb_b_len, chunk_col0=chunk_col0,
        idx_col0=idx_col0, CTOT=CTOT, ITOT=ITOT, blk_chunks=blk_chunks,
        sb_of_blk=sb_of_blk, perms=perms,
    )
    return struct, idx_np, dest_np, val_np


_NO_SPLIT = ("InstEventSemaphore", "InstDrain", "InstCollectiveCompute",
             "InstCall", "InstUnconditionalBranch", "InstConditionalBranch")


def _split_excess_waits(nc):
    """Deterministic post-pass: TRN2 instructions tolerate very few sync
    waits (walrus rejects with 'Too many sync wait commands'; Bacc's own
    generate_event_semaphores pass misses cases).  Move all but one
    semaphore wait of every ordinary instruction onto wait-only
    InstEventSemaphore instructions inserted just before it on the same
    engine (engine program order then gates the original instruction)."""
    import concourse.mybir as mybir

    for blk in nc.main_func.blocks:
        out = []
        for ins in blk.instructions:
            si = ins.sync_info
            tn = type(ins).__name__
            if si is None or tn in _NO_SPLIT or len(si.on_wait) <= 1:
                out.append(ins)
                continue
            waits = list(si.on_wait)
            keep, excess = waits[:1], waits[1:]
            while excess:
                batch, excess = excess[:2], excess[2:]
                ev = mybir.InstEventSemaphore(
                    name=nc.get_next_instruction_name(), ins=[], outs=[])
                ev.engine = ins.engine
                ev.sync_info = mybir.SyncInfo(on_wait=batch, on_update=[])
                out.append(ev)
            ins.sync_info = mybir.SyncInfo(
                on_wait=keep, on_update=list(si.on_update))
            out.append(ins)
        blk.instructions[:] = out


def _build_nc(struct, variant="full"):
    # variant: "full" | "p12" (GEMM+collective, dummy out) |
    #          "p12g" (+ gathers, dummy out)
    st = struct
    CTOT, ITOT = st["CTOT"], st["ITOT"]
    nc = bacc.Bacc("TRN2", target_bir_lowering=False, debug=False,
                   num_devices=NCORES, num_swdge_queues=NQ,
                   dynamic_dma_scratch_size=32768)
    f32 = mybir.dt.float32
    bf16 = mybir.dt.bfloat16
    i16 = mybir.dt.int16
    # table rows padded to 128 bf16 (=256B, the dma_gather minimum elem).
    # Gather time is descriptor-bound, so the pad bytes are free; bf16
    # operands let the S matmuls run at 1 cycle/row instead of fp32's 4.
    TROW = 2 * FOUT

    xt_sh = nc.dram_tensor("xt_sh", [FIN, SHARD_PAD], bf16,
                           kind="ExternalInput")
    w_in = nc.dram_tensor("w_in", [FIN, FOUT], bf16, kind="ExternalInput")
    b_rep = nc.dram_tensor("b_rep", [P, FOUT], f32, kind="ExternalInput")
    iota_in = nc.dram_tensor("iota_in", [P, P], bf16, kind="ExternalInput")
    idx_in = nc.dram_tensor("idx_in", [P, ITOT], i16, kind="ExternalInput")
    dest_in = nc.dram_tensor("dest_in", [P, CTOT], f32, kind="ExternalInput")
    val_in = nc.dram_tensor("val_in", [P, CTOT], f32, kind="ExternalInput")

    # per-quarter XW shards and gathered tables: separate tensors give the
    # Tile scheduler per-quarter deps, so AllGather q overlaps with the
    # gathers of quarter q-1 (and with the GEMM of quarters > q).
    xw_q = [nc.dram_tensor(f"xw_q{q}", [QROWS, TROW], bf16, kind="Internal")
            for q in range(NBUCKET)]
    table_q = [nc.dram_tensor(f"table_q{q}", [WINQ, TROW], bf16,
                              kind="Internal", addr_space="Shared")
               for q in range(NBUCKET)]
    out_sh = nc.dram_tensor("out_sh", [SHARD_PAD, FOUT], f32,
                            kind="ExternalOutput")

    with tile.TileContext(nc) as tc, ExitStack() as ctx:
        consts = ctx.enter_context(tc.tile_pool(name="consts", bufs=1))
        gpool = ctx.enter_context(tc.tile_pool(name="gpool", bufs=3))
        spool = ctx.enter_context(tc.tile_pool(name="spool", bufs=16))
        opool = ctx.enter_context(tc.tile_pool(name="opool", bufs=4))
        xpool = ctx.enter_context(tc.tile_pool(name="xpool", bufs=2))
        pmpool = ctx.enter_context(
            tc.tile_pool(name="pmpool", bufs=1, space="PSUM"))
        popool = ctx.enter_context(
            tc.tile_pool(name="popool", bufs=7, space="PSUM"))

        iota_t = consts.tile([P, P], bf16)
        nc.sync.dma_start(out=iota_t[:], in_=iota_in[:])
        brep_t = consts.tile([P, FOUT], f32)
        nc.sync.dma_start(out=brep_t[:], in_=b_rep[:])
        w_t = []
        for h in range(KH):
            wt = consts.tile([P, FOUT], bf16, tag=f"w{h}")
            nc.sync.dma_start(out=wt[:], in_=w_in[h * P:(h + 1) * P, :])
            w_t.append(wt)
        # edge metadata resident in SBUF for the whole kernel (loaded once,
        # so S-builds and gathers never wait on per-super-batch DMAs)
        dst_all = consts.tile([P, CTOT], f32, tag="dstall")
        nc.sync.dma_start(out=dst_all[:], in_=dest_in[:])
        vl_all = consts.tile([P, CTOT], f32, tag="vlall")
        nc.sync.dma_start(out=vl_all[:], in_=val_in[:])
        idx_all = consts.tile([P, ITOT], i16, tag="idxall")
        nc.sync.dma_start(out=idx_all[:], in_=idx_in[:])

        # pre-zero the pad half of the 4 rotating GEMM-eviction buffers once
        # (in-loop code only writes cols [:FOUT], so the pad stays zero)
        for _ in range(4):
            om = opool.tile([P, TROW], bf16, tag="om")
            nc.vector.memset(om[:, FOUT:], 0.0)

        # ------- phase 1+2: per-quarter GEMM + AllGather (pipelined) -------
        # X^T comes pre-transposed from the host, so lhsT tiles are plain
        # big strided loads (per-partition-contiguous) and PE needs no
        # transposes.  One load group + one AllGather per source quarter, so
        # quarter q's collective runs while later quarters are still in the
        # GEMM and earlier quarters are already being gathered.
        GRP = QROWS
        assert GRP % P == 0
        for g in range(SHARD_PAD // GRP):
            xts = []
            for h in range(KH):
                xt = xpool.tile([P, GRP], bf16, tag=f"xt{h}")
                nc.sync.dma_start(
                    out=xt[:], in_=xt_sh[h * P:(h + 1) * P,
                                         g * GRP:(g + 1) * GRP])
                xts.append(xt)
            for c in range(GRP // P):
                mm = pmpool.tile([P, FOUT], f32, tag="mm")
                for h in range(KH):
                    nc.tensor.matmul(
                        out=mm[:], lhsT=xts[h][:, c * P:(c + 1) * P],
                        rhs=w_t[h][:], start=(h == 0), stop=(h == KH - 1))
                om = opool.tile([P, TROW], bf16, tag="om")
                nc.vector.tensor_copy(out=om[:, :FOUT], in_=mm[:])
                nc.sync.dma_start(
                    out=xw_q[g][c * P:(c + 1) * P, :], in_=om[:])
            if variant != "p1":
                nc.gpsimd.collective_compute(
                    kind="AllGather", op=mybir.AluOpType.bypass,
                    replica_groups=[list(range(NCORES))],
                    ins=[xw_q[g][:]], outs=[table_q[g][:]],
                )

        # ---------------- phase 3: gather + segment-sum ----------------
        chunks_bb = st["chunks_bb"]
        sb_b_len = st["sb_b_len"]
        chunk_col0 = st["chunk_col0"]
        idx_col0 = st["idx_col0"]
        blk_chunks = st["blk_chunks"]
        sb_of_blk = st["sb_of_blk"]

        # SWDGE descriptor-ring throttle: chain gather k to gather k-2 so at
        # most ~2 gathers' descriptors are in flight (ring overflow wedges
        # the device otherwise; single_packet must be False for >1024 idxs).
        gather_insts = []

        if variant in ("p12", "p1"):
            obh = opool.tile([P, TROW], bf16, tag="obh")
            src = table_q[0] if variant == "p12" else xw_q[0]
            nc.sync.dma_start(out=obh[:], in_=src[:P, :])
            ob = opool.tile([P, FOUT], f32, tag="ob")
            nc.vector.tensor_copy(out=ob[:], in_=obh[:, :FOUT])
            nc.sync.dma_start(out=out_sh[:P, :], in_=ob[:])

        for sb in (range(NSB) if variant not in ("p12", "p1") else ()):
            blks = [bk for bk in range(NBLK) if sb_of_blk[bk] == sb]

            gts = [None] * NBUCKET
            for b in range(NBUCKET):
                L = int(sb_b_len[sb, b])
                if L == 0:
                    continue
                nch = L // P
                icol = int(idx_col0[sb, b])
                iw = L // 16
                gt = gpool.tile([P, nch * TROW], bf16, tag=f"g{b}")
                # quarter-gathers per (sb, bucket): keeps consumers fed early
                # while staying coarse enough to amortize the ~1us SWDGE
                # fixed cost; 4 rings + 32KB desc scratch run unthrottled.
                nA = nch
                bounds = [(0, nch)]
                for c0, c1 in bounds:
                    if c1 <= c0:
                        continue
                    gi = nc.gpsimd.dma_gather(
                        out_ap=gt[:, c0 * TROW:c1 * TROW].rearrange(
                            "p (c f) -> p c f", f=TROW),
                        in_ap=table_q[b][:, :],
                        idxs_ap=idx_all[:, icol + c0 * 8:icol + c1 * 8],
                        num_idxs=(c1 - c0) * P,
                        num_idxs_reg=(c1 - c0) * P,
                        elem_size=TROW,
                        single_packet=False,
                        queue_num=len(gather_insts) % NQ,
                    )
                    if len(gather_insts) >= GDEPTH:
                        _add_dep_helper(gi.ins, gather_insts[-GDEPTH],
                                        sync=True,
                                        reason="swdge ring throttle")
                    gather_insts.append(gi.ins)
                gts[b] = gt

            if variant == "p12g":
                continue

            for bk in blks:
                lst = blk_chunks[bk]
                if not lst:   # padding block past the real 12500 rows
                    continue
                po = popool.tile([P, FOUT], f32, tag="po")
                for k, (b, j, gcol) in enumerate(lst):
                    s_t = spool.tile([P, P], bf16, tag="s")
                    nc.vector.tensor_scalar(
                        out=s_t[:], in0=iota_t[:],
                        scalar1=dst_all[:, gcol:gcol + 1],
                        scalar2=vl_all[:, gcol:gcol + 1],
                        op0=mybir.AluOpType.is_equal,
                        op1=mybir.AluOpType.mult,
                    )
                    nc.tensor.matmul(
                        out=po[:], lhsT=s_t[:],
                        rhs=gts[b][:, j * TROW:j * TROW + FOUT],
                        start=(k == 0), stop=(k == len(lst) - 1),
                    )
                ob = opool.tile([P, FOUT], f32, tag="ob")
                nc.vector.tensor_tensor(
                    out=ob[:], in0=po[:], in1=brep_t[:],
                    op=mybir.AluOpType.add)
                nc.vector.tensor_scalar(
                    out=ob[:], in0=ob[:], scalar1=0.0, scalar2=None,
                    op0=mybir.AluOpType.max)
                nc.sync.dma_start(
                    out=out_sh[bk * P:(bk + 1) * P, :], in_=ob[:])

    nc.compile()
    _split_excess_waits(nc)
    return nc


def _prepare(X, edge_row, edge_col, edge_vals, W, b):
    """Build the compiled Bass program + per-core input maps."""
    X = np.asarray(X, dtype=np.float32)
    edge_row = np.asarray(edge_row, dtype=np.int64)
    edge_col = np.asarray(edge_col, dtype=np.int64)
    edge_vals = np.asarray(edge_vals, dtype=np.float32)
    W = np.asarray(W, dtype=np.float32)
    b = np.asarray(b, dtype=np.float32)

    global _LAST_PERMS
    struct, idx_np, dest_np, val_np = _build_plan(edge_row, edge_col, edge_vals)
    _LAST_PERMS = struct["perms"]
    nc = _build_nc(struct)

    import ml_dtypes
    b_rep = np.tile(b[None, :], (P, 1)).astype(np.float32)
    iota = np.tile(np.arange(P, dtype=np.float32)[None, :], (P, 1)).astype(
        ml_dtypes.bfloat16)

    W_bf = W.astype(ml_dtypes.bfloat16)
    # source row r lands at padded slot (r//QREAL)*QROWS + r%QREAL, so each
    # quarter holds QREAL real rows + pad (matches _build_plan's idx16)
    r = np.arange(SHARD)
    slots = (r // QREAL) * QROWS + (r % QREAL)
    in_maps = []
    for c in range(NCORES):
        xt_pad = np.zeros((FIN, SHARD_PAD), dtype=ml_dtypes.bfloat16)
        xt_pad[:, slots] = X[c * SHARD:(c + 1) * SHARD].T.astype(
            ml_dtypes.bfloat16)
        in_maps.append({
            "xt_sh": xt_pad, "w_in": W_bf, "b_rep": b_rep,
            "iota_in": iota, "idx_in": idx_np[c], "dest_in": dest_np[c],
            "val_in": val_np[c],
        })
    return nc, in_maps


def _assemble(results):
    # out_sh row perms[c][r] holds original node r (dest nodes are permuted
    # across blocks for padding balance) -- gather rows back.
    return np.concatenate(
        [results[c]["out_sh"][_LAST_PERMS[c]] for c in range(NCORES)], axis=0)


def kernel(X, edge_row, edge_col, edge_vals, W, b):
    nc, in_maps = _prepare(X, edge_row, edge_col, edge_vals, W, b)
    res = run_bass_kernel_spmd(nc, in_maps, core_ids=list(range(NCORES)))
    return _assemble(res.results)



# revision 57
# speedup vs baseline: 1.3082x; 1.3082x over previous
"""GCN layer (X@W -> edge gather/scale -> segment-sum by dest -> +b -> relu)
as a Bass/Tile kernel on 8 Trainium2 NeuronCores.

Strategy (1D node partition, SPMD single program):
  - Nodes sharded 12500/core (destination shard), padded to 100 blocks of
    128.  Each core computes its XW shard with bf16 PE matmuls; four
    per-source-quarter AllGathers replicate the bf16 XW table (rows padded
    to 256B for the gather) into every core's DRAM, pipelined so quarter
    q's collective overlaps the gathers of quarter q-1 and the GEMM of
    later quarters.
  - Edges partitioned by destination shard, bucketed by source quarter and
    grouped by dest block of 128.  Sources are gathered from the quarter
    tables with dma_gather (int16 indices, 256B rows).  The gather is
    descriptor-latency-bound, so gathers spread over all 4 SWDGE queues
    and run unthrottled (32KB descriptor scratch).
  - Dest nodes are permuted across blocks (host-side round-based greedy on
    per-quarter degree vectors) so every (block, quarter) cell fits 4
    chunks of 128 edges: padding drops from ~20% to ~4%.  The host inverts
    the permutation when assembling the output.
  - Per 128-edge chunk a selection matrix S[e, d] = val[e] * (dest[e]==d)
    is built in ONE DVE tensor_scalar op (bf16) from a constant iota tile,
    then PE computes psum[128 dests, 64] += S^T @ G (gathered bf16 rows),
    giving the multiply + segment-sum in one matmul.  +bias and relu (f32)
    on eviction.

All chunk counts are padded to the max over cores so all 8 cores run the
same program (required for the collective / PJRT SPMD launch).
"""

import math
from contextlib import ExitStack

import numpy as np

import concourse.bacc as bacc
import concourse.mybir as mybir
import concourse.tile as tile
from concourse.bass import _add_dep_helper
from concourse.bass_utils import run_bass_kernel_spmd

# Problem constants (hardcoded per contract; kernel.py must be self-contained).
N = 100000
E = 1600000
FIN = 256
FOUT = 64
NCORES = 8

P = 128                      # partitions / block size
SHARD = N // NCORES          # 12500 dest nodes per core
NBLK = 100                   # dest blocks per core (12800 rows: quarter-
                             # aligned so the AllGather splits into 4)
SHARD_PAD = NBLK * P         # 12800 (X zero-padded rows)
TABLE_ROWS = NCORES * SHARD_PAD  # 102400
NBUCKET = 4                  # edge buckets = source-shard quarters; bucket q
                             # only needs quarter-AllGather q (pipelining)
QROWS = SHARD_PAD // NBUCKET     # 3200 padded source rows per quarter
QREAL = SHARD // NBUCKET         # 3125 real source rows per quarter (the
                                 # 75-row pad sits at each quarter's end so
                                 # every bucket gets ~25% of the edges)
WINQ = NCORES * QROWS            # 25600 rows per quarter table (int16 ok)
SB_BLOCKS = 8                # dest blocks per super-batch
NSB = math.ceil(NBLK / SB_BLOCKS)
KH = FIN // P                # 2 contraction halves in the GEMM
NQ = 4                       # SWDGE descriptor queues (rings) for gathers
GDEPTH = 10000               # gather chain throttle depth (disabled: probes
                             # showed 4 rings + 64KB scratch run unthrottled)


def _build_plan(edge_row, edge_col, edge_vals):
    """Host-side edge partition/sort/pad.  Returns the uniform structure
    (shared across cores) + per-core staged arrays."""
    core = edge_row // SHARD
    r_local = edge_row - core * SHARD
    src_core = edge_col // SHARD
    src_local = edge_col - src_core * SHARD
    # bucket = source quarter (QREAL real rows each; the host stages source
    # row r at padded slot q*QROWS + (r - q*QREAL)); row index within
    # quarter table q is src_core * QROWS + that in-quarter offset
    bucket = src_local // QREAL
    idx16 = (src_core * QROWS + (src_local - bucket * QREAL)).astype(np.int16)

    # Balance (blk, bucket) cell counts by permuting each core's dest nodes
    # across blocks (greedy 4-vector LPT).  Chunk capacity is ceil(cell/128)
    # maxed over cores, so balanced cells cut the ~20% padding to a few %.
    # The host inverts the permutation in _assemble; sources are unaffected.
    perms = np.empty((NCORES, SHARD), dtype=np.int64)
    for c in range(NCORES):
        m = core == c
        deg4 = np.zeros((SHARD, NBUCKET), dtype=np.int64)
        np.add.at(deg4, (r_local[m], bucket[m]), 1)
        order = np.argsort(-deg4.sum(axis=1), kind="stable")
        cells = np.zeros((NBLK, NBUCKET), dtype=np.int64)
        fills = np.zeros(NBLK, dtype=np.int64)
        slot = np.empty(SHARD, dtype=np.int64)
        big = 1 << 30
        cap = 4 * P  # hard cell cap: exactly 4 chunks per (blk, bucket)
        rate = deg4.sum(axis=0) / SHARD  # per-bucket mean node degree
        # rounds of NBLK nodes, one per block: fills stay exactly even and
        # every block's bucket profile tracks the uniform trajectory
        for r0 in range(0, SHARD, NBLK):
            rnd = order[r0:r0 + NBLK]
            used = np.zeros(NBLK, dtype=bool)
            ideal = np.outer(fills + 1, rate)
            for n in rnd:
                after = cells + deg4[n]
                cost = ((after - ideal) ** 2).sum(axis=1)
                cost = cost + (after > cap).any(axis=1) * big + used * (4 * big)
                j = int(cost.argmin())
                cells[j] += deg4[n]
                slot[n] = j * P + fills[j]
                fills[j] += 1
                used[j] = True
        perms[c] = slot
    new_r = perms[core, r_local]
    blk = new_r // P
    dest_in_blk = (new_r - blk * P).astype(np.float32)

    # sort edges by (core, blk, bucket)
    order = np.lexsort((bucket, blk, core))
    core_s = core[order]
    blk_s = blk[order]
    bucket_s = bucket[order]
    idx16_s = idx16[order]
    dest_s = dest_in_blk[order]
    val_s = edge_vals[order].astype(np.float32)

    # segment counts per (core, blk, bucket)
    seg_key = (core_s * NBLK + blk_s) * NBUCKET + bucket_s
    counts = np.bincount(seg_key, minlength=NCORES * NBLK * NBUCKET).reshape(
        NCORES, NBLK, NBUCKET
    )
    # uniform capacity (in chunks of 128 edges) per (blk, bucket): max over cores
    chunks_bb = np.ceil(counts / P).astype(np.int64).max(axis=0)  # [NBLK, NBUCKET]
    # blocks holding real output rows need >=1 chunk so their PSUM is reset;
    # pure-padding blocks (rows >= SHARD, dropped by _assemble) may be empty
    # and are skipped in the kernel.
    nblk_real = math.ceil(SHARD / P)
    assert chunks_bb[:nblk_real].sum(axis=1).min() >= 1
    cap_bb = chunks_bb * P

    # ---- static layout ----
    # stream order: (sb, bucket, blk in sb, chunk)
    sb_of_blk = np.arange(NBLK) // SB_BLOCKS
    # chunk columns per (sb, bucket): sum of chunks of its blocks
    # slot offsets for each (blk, bucket) within its (sb, bucket) stream
    slot_off = np.zeros((NBLK, NBUCKET), dtype=np.int64)
    sb_b_len = np.zeros((NSB, NBUCKET), dtype=np.int64)   # slots per (sb, bucket)
    for sb in range(NSB):
        blks = np.where(sb_of_blk == sb)[0]
        for b in range(NBUCKET):
            off = 0
            for bk in blks:
                slot_off[bk, b] = off
                off += cap_bb[bk, b]
            sb_b_len[sb, b] = off
    # global offsets: chunk columns and idx columns per (sb, bucket)
    chunk_col0 = np.zeros((NSB, NBUCKET), dtype=np.int64)
    idx_col0 = np.zeros((NSB, NBUCKET), dtype=np.int64)
    ccur = icur = 0
    for sb in range(NSB):
        for b in range(NBUCKET):
            chunk_col0[sb, b] = ccur
            idx_col0[sb, b] = icur
            ccur += sb_b_len[sb, b] // P
            icur += sb_b_len[sb, b] // 16
    CTOT = ccur   # total chunks per core
    ITOT = icur   # total idx columns per core

    # global slot index for every edge:
    #   slot = global_slot0[sb,b] + slot_off[blk,b] + rank_within_segment
    # where global_slot0 = chunk_col0 * 128
    first_of_seg = np.zeros(NCORES * NBLK * NBUCKET + 1, dtype=np.int64)
    np.cumsum(counts.reshape(-1), out=first_of_seg[1:])
    rank = np.arange(len(core_s)) - first_of_seg[seg_key]
    slot = (
        chunk_col0[sb_of_blk[blk_s], bucket_s] * P
        + slot_off[blk_s, bucket_s]
        + rank
    )

    # ---- per-core staged arrays ----
    idx_streams = np.zeros((NCORES, CTOT * P), dtype=np.int16)
    dest_streams = np.zeros((NCORES, CTOT * P), dtype=np.float32)
    val_streams = np.zeros((NCORES, CTOT * P), dtype=np.float32)
    for c in range(NCORES):
        m = core_s == c
        idx_streams[c, slot[m]] = idx16_s[m]
        dest_streams[c, slot[m]] = dest_s[m]
        val_streams[c, slot[m]] = val_s[m]

    # dest/val DRAM layout [128, CTOT]: chunk j, partition p <- stream[j*128+p]
    dest_np = dest_streams.reshape(NCORES, CTOT, P).transpose(0, 2, 1).copy()
    val_np = val_streams.reshape(NCORES, CTOT, P).transpose(0, 2, 1).copy()

    # idx DRAM layout [128, ITOT] int16: within each (sb,b) segment of the
    # stream, idx i -> partition i%16 (replicated over the 8 groups of 16),
    # column i//16
    idx_np = np.zeros((NCORES, P, ITOT), dtype=np.int16)
    for sb in range(NSB):
        for b in range(NBUCKET):
            L = int(sb_b_len[sb, b])
            if L == 0:
                continue
            s0 = int(chunk_col0[sb, b]) * P
            i0 = int(idx_col0[sb, b])
            seg = idx_streams[:, s0:s0 + L].reshape(NCORES, L // 16, 16)
            seg = seg.transpose(0, 2, 1)  # [NCORES, 16, L//16]
            idx_np[:, :, i0:i0 + L // 16] = np.tile(seg, (1, 8, 1))

    # per-block chunk list: (bucket, j_local_in_gather, global_chunk_col)
    blk_chunks = []
    for bk in range(NBLK):
        sb = int(sb_of_blk[bk])
        lst = []
        for b in range(NBUCKET):
            nch = int(chunks_bb[bk, b])
            j0 = int(slot_off[bk, b]) // P
            c0 = int(chunk_col0[sb, b]) + j0
            for k in range(nch):
                lst.append((b, j0 + k, c0 + k))
        blk_chunks.append(lst)

    struct = dict(
        chunks_bb=chunks_bb, sb_b_len=sb_b_len, chunk_col0=chunk_col0,
        idx_col0=idx_col0, CTOT=CTOT, ITOT=ITOT, blk_chunks=blk_chunks,
        sb_of_blk=sb_of_blk, perms=perms,
    )
    return struct, idx_np, dest_np, val_np


_NO_SPLIT = ("InstEventSemaphore", "InstDrain", "InstCollectiveCompute",
             "InstCall", "InstUnconditionalBranch", "InstConditionalBranch")


def _split_excess_waits(nc):
    """Deterministic post-pass: TRN2 instructions tolerate very few sync
    waits (walrus rejects with 'Too many sync wait commands'; Bacc's own
    generate_event_semaphores pass misses cases).  Move all but one
    semaphore wait of every ordinary instruction onto wait-only
    InstEventSemaphore instructions inserted just before it on the same
    engine (engine program order then gates the original instruction)."""
    import concourse.mybir as mybir

    for blk in nc.main_func.blocks:
        out = []
        for ins in blk.instructions:
            si = ins.sync_info
            tn = type(ins).__name__
            if si is None or tn in _NO_SPLIT or len(si.on_wait) <= 1:
                out.append(ins)
                continue
            waits = list(si.on_wait)
            keep, excess = waits[:1], waits[1:]
            while excess:
                batch, excess = excess[:2], excess[2:]
                ev = mybir.InstEventSemaphore(
                    name=nc.get_next_instruction_name(), ins=[], outs=[])
                ev.engine = ins.engine
                ev.sync_info = mybir.SyncInfo(on_wait=batch, on_update=[])
                out.append(ev)
            ins.sync_info = mybir.SyncInfo(
                on_wait=keep, on_update=list(si.on_update))
            out.append(ins)
        blk.instructions[:] = out


def _build_nc(struct, variant="full"):
    # variant: "full" | "p12" (GEMM+collective, dummy out) |
    #          "p12g" (+ gathers, dummy out)
    st = struct
    CTOT, ITOT = st["CTOT"], st["ITOT"]
    nc = bacc.Bacc("TRN2", target_bir_lowering=False, debug=False,
                   num_devices=NCORES, num_swdge_queues=NQ,
                   dynamic_dma_scratch_size=32768)
    f32 = mybir.dt.float32
    bf16 = mybir.dt.bfloat16
    i16 = mybir.dt.int16
    # table rows padded to 128 bf16 (=256B, the dma_gather minimum elem).
    # Gather time is descriptor-bound, so the pad bytes are free; bf16
    # operands let the S matmuls run at 1 cycle/row instead of fp32's 4.
    TROW = 2 * FOUT

    xt_sh = nc.dram_tensor("xt_sh", [FIN, SHARD_PAD], bf16,
                           kind="ExternalInput")
    w_in = nc.dram_tensor("w_in", [FIN, FOUT], bf16, kind="ExternalInput")
    b_rep = nc.dram_tensor("b_rep", [P, FOUT], f32, kind="ExternalInput")
    iota_in = nc.dram_tensor("iota_in", [P, P], bf16, kind="ExternalInput")
    idx_in = nc.dram_tensor("idx_in", [P, ITOT], i16, kind="ExternalInput")
    dest_in = nc.dram_tensor("dest_in", [P, CTOT], f32, kind="ExternalInput")
    val_in = nc.dram_tensor("val_in", [P, CTOT], f32, kind="ExternalInput")

    # per-quarter XW shards and gathered tables: separate tensors give the
    # Tile scheduler per-quarter deps, so AllGather q overlaps with the
    # gathers of quarter q-1 (and with the GEMM of quarters > q).
    xw_q = [nc.dram_tensor(f"xw_q{q}", [QROWS, TROW], bf16, kind="Internal")
            for q in range(NBUCKET)]
    table_q = [nc.dram_tensor(f"table_q{q}", [WINQ, TROW], bf16,
                              kind="Internal", addr_space="Shared")
               for q in range(NBUCKET)]
    out_sh = nc.dram_tensor("out_sh", [SHARD_PAD, FOUT], f32,
                            kind="ExternalOutput")

    with tile.TileContext(nc) as tc, ExitStack() as ctx:
        consts = ctx.enter_context(tc.tile_pool(name="consts", bufs=1))
        gpool = ctx.enter_context(tc.tile_pool(name="gpool", bufs=3))
        spool = ctx.enter_context(tc.tile_pool(name="spool", bufs=16))
        opool = ctx.enter_context(tc.tile_pool(name="opool", bufs=4))
        xpool = ctx.enter_context(tc.tile_pool(name="xpool", bufs=2))
        pmpool = ctx.enter_context(
            tc.tile_pool(name="pmpool", bufs=1, space="PSUM"))
        popool = ctx.enter_context(
            tc.tile_pool(name="popool", bufs=7, space="PSUM"))

        iota_t = consts.tile([P, P], bf16)
        nc.sync.dma_start(out=iota_t[:], in_=iota_in[:])
        brep_t = consts.tile([P, FOUT], f32)
        nc.sync.dma_start(out=brep_t[:], in_=b_rep[:])
        w_t = []
        for h in range(KH):
            wt = consts.tile([P, FOUT], bf16, tag=f"w{h}")
            nc.sync.dma_start(out=wt[:], in_=w_in[h * P:(h + 1) * P, :])
            w_t.append(wt)
        # edge metadata resident in SBUF for the whole kernel (loaded once,
        # so S-builds and gathers never wait on per-super-batch DMAs)
        dst_all = consts.tile([P, CTOT], f32, tag="dstall")
        nc.sync.dma_start(out=dst_all[:], in_=dest_in[:])
        vl_all = consts.tile([P, CTOT], f32, tag="vlall")
        nc.sync.dma_start(out=vl_all[:], in_=val_in[:])
        idx_all = consts.tile([P, ITOT], i16, tag="idxall")
        nc.sync.dma_start(out=idx_all[:], in_=idx_in[:])

        # pre-zero the pad half of the 4 rotating GEMM-eviction buffers once
        # (in-loop code only writes cols [:FOUT], so the pad stays zero)
        for _ in range(4):
            om = opool.tile([P, TROW], bf16, tag="om")
            nc.vector.memset(om[:, FOUT:], 0.0)

        # ------- phase 1+2: per-quarter GEMM + AllGather (pipelined) -------
        # X^T comes pre-transposed from the host, so lhsT tiles are plain
        # big strided loads (per-partition-contiguous) and PE needs no
        # transposes.  One load group + one AllGather per source quarter, so
        # quarter q's collective runs while later quarters are still in the
        # GEMM and earlier quarters are already being gathered.
        GRP = QROWS
        assert GRP % P == 0
        for g in range(SHARD_PAD // GRP):
            xts = []
            for h in range(KH):
                xt = xpool.tile([P, GRP], bf16, tag=f"xt{h}")
                nc.sync.dma_start(
                    out=xt[:], in_=xt_sh[h * P:(h + 1) * P,
                                         g * GRP:(g + 1) * GRP])
                xts.append(xt)
            for c in range(GRP // P):
                mm = pmpool.tile([P, FOUT], f32, tag="mm")
                for h in range(KH):
                    nc.tensor.matmul(
                        out=mm[:], lhsT=xts[h][:, c * P:(c + 1) * P],
                        rhs=w_t[h][:], start=(h == 0), stop=(h == KH - 1))
                om = opool.tile([P, TROW], bf16, tag="om")
                nc.vector.tensor_copy(out=om[:, :FOUT], in_=mm[:])
                nc.sync.dma_start(
                    out=xw_q[g][c * P:(c + 1) * P, :], in_=om[:])
            if variant != "p1":
                nc.gpsimd.collective_compute(
                    kind="AllGather", op=mybir.AluOpType.bypass,
                    replica_groups=[list(range(NCORES))],
                    ins=[xw_q[g][:]], outs=[table_q[g][:]],
                )

        # ---------------- phase 3: gather + segment-sum ----------------
        chunks_bb = st["chunks_bb"]
        sb_b_len = st["sb_b_len"]
        chunk_col0 = st["chunk_col0"]
        idx_col0 = st["idx_col0"]
        blk_chunks = st["blk_chunks"]
        sb_of_blk = st["sb_of_blk"]

        # SWDGE descriptor-ring throttle: chain gather k to gather k-2 so at
        # most ~2 gathers' descriptors are in flight (ring overflow wedges
        # the device otherwise; single_packet must be False for >1024 idxs).
        gather_insts = []

        if variant in ("p12", "p1"):
            obh = opool.tile([P, TROW], bf16, tag="obh")
            src = table_q[0] if variant == "p12" else xw_q[0]
            nc.sync.dma_start(out=obh[:], in_=src[:P, :])
            ob = opool.tile([P, FOUT], f32, tag="ob")
            nc.vector.tensor_copy(out=ob[:], in_=obh[:, :FOUT])
            nc.sync.dma_start(out=out_sh[:P, :], in_=ob[:])

        for sb in (range(NSB) if variant not in ("p12", "p1") else ()):
            blks = [bk for bk in range(NBLK) if sb_of_blk[bk] == sb]

            gts = [None] * NBUCKET
            for b in range(NBUCKET):
                L = int(sb_b_len[sb, b])
                if L == 0:
                    continue
                nch = L // P
                icol = int(idx_col0[sb, b])
                iw = L // 16
                gt = gpool.tile([P, nch * TROW], bf16, tag=f"g{b}")
                # quarter-gathers per (sb, bucket): keeps consumers fed early
                # while staying coarse enough to amortize the ~1us SWDGE
                # fixed cost; 4 rings + 32KB desc scratch run unthrottled.
                nA = (nch + 1) // 2
                bounds = [(i * nA, min((i + 1) * nA, nch)) for i in range(2)]
                for c0, c1 in bounds:
                    if c1 <= c0:
                        continue
                    gi = nc.gpsimd.dma_gather(
                        out_ap=gt[:, c0 * TROW:c1 * TROW].rearrange(
                            "p (c f) -> p c f", f=TROW),
                        in_ap=table_q[b][:, :],
                        idxs_ap=idx_all[:, icol + c0 * 8:icol + c1 * 8],
                        num_idxs=(c1 - c0) * P,
                        num_idxs_reg=(c1 - c0) * P,
                        elem_size=TROW,
                        single_packet=False,
                        queue_num=len(gather_insts) % NQ,
                    )
                    if len(gather_insts) >= GDEPTH:
                        _add_dep_helper(gi.ins, gather_insts[-GDEPTH],
                                        sync=True,
                                        reason="swdge ring throttle")
                    gather_insts.append(gi.ins)
                gts[b] = gt

            if variant == "p12g":
                continue

            for bk in blks:
                lst = blk_chunks[bk]
                if not lst:   # padding block past the real 12500 rows
                    continue
                po = popool.tile([P, FOUT], f32, tag="po")
                for k, (b, j, gcol) in enumerate(lst):
                    s_t = spool.tile([P, P], bf16, tag="s")
                    nc.vector.tensor_scalar(
                        out=s_t[:], in0=iota_t[:],
                        scalar1=dst_all[:, gcol:gcol + 1],
                        scalar2=vl_all[:, gcol:gcol + 1],
                        op0=mybir.AluOpType.is_equal,
                        op1=mybir.AluOpType.mult,
                    )
                    nc.tensor.matmul(
                        out=po[:], lhsT=s_t[:],
                        rhs=gts[b][:, j * TROW:j * TROW + FOUT],
                        start=(k == 0), stop=(k == len(lst) - 1),
                    )
                ob = opool.tile([P, FOUT], f32, tag="ob")
                nc.vector.tensor_tensor(
                    out=ob[:], in0=po[:], in1=brep_t[:],
                    op=mybir.AluOpType.add)
                nc.vector.tensor_scalar(
                    out=ob[:], in0=ob[:], scalar1=0.0, scalar2=None,
                    op0=mybir.AluOpType.max)
                nc.sync.dma_start(
                    out=out_sh[bk * P:(bk + 1) * P, :], in_=ob[:])

    nc.compile()
    _split_excess_waits(nc)
    return nc


def _prepare(X, edge_row, edge_col, edge_vals, W, b):
    """Build the compiled Bass program + per-core input maps."""
    X = np.asarray(X, dtype=np.float32)
    edge_row = np.asarray(edge_row, dtype=np.int64)
    edge_col = np.asarray(edge_col, dtype=np.int64)
    edge_vals = np.asarray(edge_vals, dtype=np.float32)
    W = np.asarray(W, dtype=np.float32)
    b = np.asarray(b, dtype=np.float32)

    global _LAST_PERMS
    struct, idx_np, dest_np, val_np = _build_plan(edge_row, edge_col, edge_vals)
    _LAST_PERMS = struct["perms"]
    nc = _build_nc(struct)

    import ml_dtypes
    b_rep = np.tile(b[None, :], (P, 1)).astype(np.float32)
    iota = np.tile(np.arange(P, dtype=np.float32)[None, :], (P, 1)).astype(
        ml_dtypes.bfloat16)

    W_bf = W.astype(ml_dtypes.bfloat16)
    # source row r lands at padded slot (r//QREAL)*QROWS + r%QREAL, so each
    # quarter holds QREAL real rows + pad (matches _build_plan's idx16)
    r = np.arange(SHARD)
    slots = (r // QREAL) * QROWS + (r % QREAL)
    in_maps = []
    for c in range(NCORES):
        xt_pad = np.zeros((FIN, SHARD_PAD), dtype=ml_dtypes.bfloat16)
        xt_pad[:, slots] = X[c * SHARD:(c + 1) * SHARD].T.astype(
            ml_dtypes.bfloat16)
        in_maps.append({
            "xt_sh": xt_pad, "w_in": W_bf, "b_rep": b_rep,
            "iota_in": iota, "idx_in": idx_np[c], "dest_in": dest_np[c],
            "val_in": val_np[c],
        })
    return nc, in_maps


def _assemble(results):
    # out_sh row perms[c][r] holds original node r (dest nodes are permuted
    # across blocks for padding balance) -- gather rows back.
    return np.concatenate(
        [results[c]["out_sh"][_LAST_PERMS[c]] for c in range(NCORES)], axis=0)


def kernel(X, edge_row, edge_col, edge_vals, W, b):
    nc, in_maps = _prepare(X, edge_row, edge_col, edge_vals, W, b)
    res = run_bass_kernel_spmd(nc, in_maps, core_ids=list(range(NCORES)))
    return _assemble(res.results)

